# revision 12
# baseline (speedup 1.0000x reference)
"""Trainium2 Bass kernel for GroupNorm -> self-attention -> proj + residual.

v2 design (per image, b=32 total, data-parallel over 8 cores):
    xn    = GroupNorm(x, 8 groups, affine)                    [c=256, n=1024]
    Host folds the projections algebraically:
      G  = Wk^T Wq          st[m,n] = xn_m^T G xn_n  (q/k matmuls fused)
      Hu = [Wo Wv; u^T]     v'[m,:] = H xn_m,  r_m = u.xn_m (u = Wk^T bq)
      fb = Wo bv + out_b    (folded into the GroupNorm residual tile)
    The per-query score-bias terms cancel in softmax; the per-key term
    r_m rides as an extra output column of the v' matmul and folds into
    the exp bias.  exp is shifted by -3 so est fits fp8e4 (max ~107<240).
    est and v' are stored fp8e4 -> attn.v and the denominator column-sum
    run as DoubleRow matmuls (2x PE rate); the output projection is gone
    (folded into v'), so attn.v produces the final pre-residual output.

Engine split: PE matmuls; ACT t-copyback + exp; DVE stats/approx-recip/
copies/psum-normalize; Pool (GPSIMD) GroupNorm applies + residual adds
(the m=1 adds ride DVE so the tail drains on two engines).  Matmul inputs
fp16 (t, xn, G, Hu) and fp8e4 (est, v'); accumulation fp32.

HW-measured notes (axon trn2, wall-clock rep-differencing):
  - baseline (q/k/v + proj matmuls, bf16, full DVE reciprocal): 172.1 us
  - this kernel: ~120-127 us per 4-image iteration; rel err 1.27e-2.
  - DVE `reciprocal` costs ~6.7 cyc/elem on HW (5x the cost model);
    `reciprocal_approx_fast` (~18 bits) measured ~5x faster end-to-end.
  - fp8 DoubleRow MM (N=512) ~234 ns vs bf16 ~252 ns per instruction
    (2.15x per unit contraction) - model undercharges DR at 107 ns.
  - For_i loop overhead (barrier + back-edge) is only ~1.7 us/iter;
    unrolling 2 bodies/iteration measured SLOWER (instruction fetch).
"""

import numpy as np
from contextlib import ExitStack

import concourse.bass as bass
import concourse.tile as tile
import concourse.mybir as mybir
from concourse import bacc
from concourse.bass import ts
from concourse.bass_utils import run_bass_kernel_spmd

P = 128
N_CORES = 8
B, C, H, W = 32, 256, 32, 32
N = H * W                      # 1024 pixels
IMGS = B // N_CORES            # 4 images per core
NH = C // P                    # 2 channel halves
NT = N // P                    # 8 pixel tiles
GROUPS = 8
EPS = 1e-5
F32 = mybir.dt.float32
F16 = mybir.dt.float16
F8 = mybir.dt.float8e4
AF = mybir.ActivationFunctionType
OP = mybir.AluOpType
DR = mybir.MatmulPerfMode.DoubleRow
CHUNK = 512                    # matmul moving free dim (one PSUM bank f32)
NCH = N // CHUNK               # 2 chunks
SHIFT = -3.0                   # exp shift: est = exp(st/16 - 3), cancels in ratio

PHASE_OF = {}


class _phase:
    """Records which instructions each phase emits (for trace attribution)."""

    def __init__(self, nc, name):
        self.nc, self.name = nc, name

    def __enter__(self):
        self.before = set(self.nc.inst_map)
        return self

    def __exit__(self, *a):
        for n in set(self.nc.inst_map) - self.before:
            PHASE_OF[n] = self.name


def _emit(ctx: ExitStack, tc: tile.TileContext, t: dict, reps: int = 1):
    nc = tc.nc

    singles = ctx.enter_context(tc.tile_pool(name="singles", bufs=1))
    p_x = ctx.enter_context(tc.tile_pool(name="p_x", bufs=3))
    p_stats = ctx.enter_context(tc.tile_pool(name="p_stats", bufs=4))
    p_xnb = ctx.enter_context(tc.tile_pool(name="p_xnb", bufs=3))
    p_xnfb = ctx.enter_context(tc.tile_pool(name="p_xnfb", bufs=3))
    p_t = ctx.enter_context(tc.tile_pool(name="p_t", bufs=3))
    p_vt = ctx.enter_context(tc.tile_pool(name="p_vt", bufs=3))
    p_r2 = ctx.enter_context(tc.tile_pool(name="p_r2", bufs=3))
    p_est = ctx.enter_context(tc.tile_pool(name="p_est", bufs=3))
    p_recip = ctx.enter_context(tc.tile_pool(name="p_recip", bufs=3))
    p_tmp = ctx.enter_context(tc.tile_pool(name="p_tmp", bufs=6))
    p_fin = ctx.enter_context(tc.tile_pool(name="p_fin", bufs=3))
    ps_big = ctx.enter_context(tc.tile_pool(name="ps_big", bufs=3, space="PSUM"))
    ps_sm = ctx.enter_context(tc.tile_pool(name="ps_sm", bufs=1, space="PSUM"))

    # ---- load constants / weights into SBUF once ----
    s_gT = singles.tile([P, NH, C], F16)
    nc.sync.dma_start(s_gT[:], t["gT"].rearrange("h p o -> p h o"))
    s_huT = singles.tile([P, NH, C + 1], F16)
    nc.sync.dma_start(s_huT[:], t["huT"].rearrange("h p o -> p h o"))
    s_gnw = singles.tile([P, NH], F32)
    nc.sync.dma_start(s_gnw[:], t["gnw"].rearrange("h p -> p h"))
    s_gnbfb = singles.tile([P, NH, 2], F32)  # col0 = gn_b, col1 = gn_b + fb
    nc.sync.dma_start(s_gnbfb[:], t["gnbfb"].rearrange("h p k -> p h k"))
    s_ind = singles.tile([P, NH, GROUPS], F32)
    nc.sync.dma_start(s_ind[:], t["ind"].rearrange("h p g -> p h g"))
    s_indT = singles.tile([GROUPS, NH, P], F32)
    nc.sync.dma_start(s_indT[:], t["indT"])
    s_ones8 = singles.tile([P, 2, P], F8)
    nc.vector.memset(s_ones8[:], 1.0)
    s_ones16 = singles.tile([P, P], F16)
    nc.vector.memset(s_ones16[:], 1.0)

    # PE warmup: dense dummy matmuls during the GroupNorm head so the HAM
    # clock-gate reaches 8/8 before the real matmuls start (HW-only effect).
    ps_w = ps_sm.tile([P, C + 1], F32, tag="vt")
    for _ in range(10):
        nc.tensor.matmul(ps_w[:, 0:C], s_ones16[:], s_gT[:, 0, :],
                         start=True, stop=True)
    w_sink = p_stats.tile([1, 1], F32, tag="wsink")
    nc.vector.tensor_copy(w_sink[:], ps_w[0:1, 0:1])

    x_ap = t["x"]       # [IMGS, NH, P, N]
    out_ap = t["out"]   # [IMGS, NH, P, N]

    def body():
        _body(nc, tc, t, pools)

    pools = dict(p_x=p_x, p_stats=p_stats, p_xnb=p_xnb, p_xnfb=p_xnfb,
                 p_t=p_t, p_vt=p_vt, p_r2=p_r2, p_est=p_est,
                 p_recip=p_recip, p_tmp=p_tmp, p_fin=p_fin, ps_big=ps_big,
                 ps_sm=ps_sm, s_gT=s_gT, s_huT=s_huT, s_gnw=s_gnw,
                 s_gnbfb=s_gnbfb, s_ind=s_ind, s_indT=s_indT,
                 s_ones8=s_ones8, x_ap=x_ap, out_ap=out_ap)

    # One body per hardware-loop iteration. (Unrolling 2 bodies/iteration was
    # tried and measured SLOWER on HW — the doubled instruction stream costs
    # more in sequencer fetch than the amortized barrier saves.)
    if reps > 1:
        with tc.For_i(0, reps, 1, hint_engines=(mybir.EngineType.PE,)):
            body()
    else:
        body()


def _body(nc, tc, t, pl):
    p_x, p_stats, p_xnb = pl["p_x"], pl["p_stats"], pl["p_xnb"]
    p_xnfb, p_t, p_vt = pl["p_xnfb"], pl["p_t"], pl["p_vt"]
    p_r2, p_est, p_recip = pl["p_r2"], pl["p_est"], pl["p_recip"]
    p_tmp, p_fin, ps_big, ps_sm = pl["p_tmp"], pl["p_fin"], pl["ps_big"], pl["ps_sm"]
    s_gT, s_huT, s_gnw = pl["s_gT"], pl["s_huT"], pl["s_gnw"]
    s_gnbfb, s_ind, s_indT = pl["s_gnbfb"], pl["s_ind"], pl["s_indT"]
    s_ones8, x_ap, out_ap = pl["s_ones8"], pl["x_ap"], pl["out_ap"]

    for img in range(IMGS):
        with _phase(nc, "gn"), tc.high_priority(offset=120):
    # ---------------- GroupNorm (fully per-half: groups never span halves) ---
    # high_priority: img i+1's GN (DMA/stats/applies) must not queue behind
    # img i's attention epilogue on Pool/DVE, or the next image's matmuls
    # stall waiting for xnb.
            x_t = p_x.tile([P, NH, N], F32, tag="x")
            xnb = p_xnb.tile([P, NH, N], F16, tag="xnb")
            xnfb = p_xnfb.tile([P, NH, N], F32, tag="xnfb")
            for h in range(NH):
                for s in range(2):
                    nc.sync.dma_start(x_t[:, h, ts(s, CHUNK)],
                                      x_ap[img, h, :, ts(s, CHUNK)])

                # per-channel mean / E[x^2] via bn_stats (free dim cap 512)
                st6 = p_stats.tile([P, 2, 6], F32, tag="st6")
                xv = x_t[:, h].rearrange("p (s f) -> p s f", f=512)
                for s in range(2):
                    nc.vector.bn_stats(out=st6[:, s, :], in_=xv[:, s, :])
                mv = p_stats.tile([P, 2], F32, tag="mv")
                nc.vector.bn_aggr(out=mv[:], in_=st6[:])
                mm = p_stats.tile([P, 2], F32, tag="mm")  # (mean, E[x^2])
                nc.vector.tensor_copy(mm[:, 0:1], mv[:, 0:1])
                nc.vector.tensor_tensor(mm[:, 1:2], mv[:, 0:1], mv[:, 0:1], OP.mult)
                nc.vector.tensor_tensor(mm[:, 1:2], mm[:, 1:2], mv[:, 1:2], OP.add)

                # this half's 4 group stats: [4, 2] = ind_h.T @ mm
                psg = ps_sm.tile([4, 2], F32, tag="gn")
                nc.tensor.matmul(psg[:], s_ind[:, h, :4], mm[:],
                                 start=True, stop=True)
                grp = p_stats.tile([4, 2], F32, tag="grp")  # (mu, rstd)
                nc.vector.tensor_copy(grp[:, 0:1], psg[:, 0:1])
                nc.vector.tensor_copy(grp[:, 1:2], psg[:, 1:2])
                v = p_stats.tile([4, 3], F32, tag="musq")  # var+eps, s, t
                nc.vector.tensor_tensor(v[:, 1:2], grp[:, 0:1], grp[:, 0:1], OP.mult)
                nc.vector.tensor_tensor(v[:, 0:1], grp[:, 1:2], v[:, 1:2], OP.subtract)
                nc.vector.tensor_scalar(out=v[:, 0:1], in0=v[:, 0:1], scalar1=EPS,
                                        scalar2=None, op0=OP.add)
                # rstd = 1/sqrt(v) by Newton on sqrt from s0=1 (group var ~ 1),
                # all on DVE — keeps ACT's table set pinned to exp.
                # s <- 0.5*(s + v/s), twice; then rstd = 1/s.
                nc.vector.tensor_scalar(out=v[:, 1:2], in0=v[:, 0:1], scalar1=1.0,
                                        scalar2=0.5, op0=OP.add, op1=OP.mult)
                for _ in range(2):
                    nc.vector.reciprocal(v[:, 2:3], v[:, 1:2])
                    nc.vector.tensor_tensor(v[:, 2:3], v[:, 0:1], v[:, 2:3], OP.mult)
                    nc.vector.tensor_tensor(v[:, 1:2], v[:, 1:2], v[:, 2:3], OP.add)
                    nc.vector.tensor_scalar(out=v[:, 1:2], in0=v[:, 1:2],
                                            scalar1=0.5, scalar2=None, op0=OP.mult)
                nc.vector.reciprocal(grp[:, 1:2], v[:, 1:2])

                # broadcast 4 group (mu, rstd) to this half's 128 channels
                psb = ps_sm.tile([P, 2], F32, tag="gn")
                nc.tensor.matmul(psb[:], s_indT[:4, h, :], grp[:],
                                 start=True, stop=True)
                ab = p_stats.tile([P, 3], F32, tag="ab")  # a, b, b+fb
                a = ab[:, 0:1]
                nc.vector.tensor_tensor(a, psb[:, 1:2], s_gnw[:, h:h + 1], OP.mult)
                mua = ab[:, 1:2]
                nc.vector.tensor_tensor(mua, psb[:, 0:1], a, OP.mult)
                # b = gn_b - mu*a ; b_fb = (gn_b + fb) - mu*a
                nc.vector.tensor_tensor(ab[:, 2:3], s_gnbfb[:, h, 1:2], mua, OP.subtract)
                nc.vector.tensor_tensor(mua, s_gnbfb[:, h, 0:1], mua, OP.subtract)

                # apply on GPSIMD: xnb = fp16(x*a+b); xnfb = f32(x*a+(b+fb))
                # (per-chunk so downstream matmuls can start on chunk 0)
                for s in range(2):
                    nc.gpsimd.tensor_scalar(out=xnb[:, h, ts(s, CHUNK)],
                                            in0=x_t[:, h, ts(s, CHUNK)],
                                            scalar1=ab[:, 0:1], scalar2=ab[:, 1:2],
                                            op0=OP.mult, op1=OP.add)
                for s in range(2):
                    nc.gpsimd.tensor_scalar(out=xnfb[:, h, ts(s, CHUNK)],
                                            in0=x_t[:, h, ts(s, CHUNK)],
                                            scalar1=ab[:, 0:1], scalar2=ab[:, 2:3],
                                            op0=OP.mult, op1=OP.add)

        with _phase(nc, "t"):
    # ---------------- t = G @ xn  (q/k fused; [c, n] layout) ----------------
            t_t = p_t.tile([P, NH, N], F16, tag="t")
            for j in range(NH):
                ps = ps_big.tile([P, N], F32, tag="big")
                for h in range(NH):
                    for ch in range(NCH):
                        nc.tensor.matmul(ps[:, ts(ch, CHUNK)],
                                         s_gT[:, h, ts(j, P)],
                                         xnb[:, h, ts(ch, CHUNK)],
                                         start=(h == 0), stop=(h == NH - 1))
                nc.scalar.activation(out=t_t[:, j], in_=ps[:], func=AF.Copy)

        with _phase(nc, "vt"):
    # v' in [n, c] layout via lhsT=xnb; extra col 256 = r_m = u.xn_m
            vt = p_vt.tile([P, NT, C], F8, tag="vt")
            r2 = p_r2.tile([P, NT], F32, tag="r2")   # exp bias: r/16 + SHIFT
            for tt in range(NT):
                ps2 = ps_sm.tile([P, C + 1], F32, tag="vt")
                for h in range(NH):
                    nc.tensor.matmul(ps2[:], xnb[:, h, ts(tt, P)],
                                     s_huT[:, h, :],
                                     start=(h == 0), stop=(h == NH - 1))
                nc.vector.tensor_copy(out=vt[:, tt], in_=ps2[:, 0:C])
                nc.vector.tensor_scalar(out=r2[:, tt:tt + 1],
                                        in0=ps2[:, C:C + 1],
                                        scalar1=1.0 / 16.0, scalar2=SHIFT,
                                        op0=OP.mult, op1=OP.add)

        with _phase(nc, "scores"):
    # ---------------- scores^T (st[m,n] = xn_m.G.xn_n) + exp ----------------
            est = p_est.tile([P, NT, N], F8, tag="est")
            for tt in range(NT):
                ps = ps_big.tile([P, N], F32, tag="big")
                for h in range(NH):
                    for ch in range(NCH):
                        nc.tensor.matmul(ps[:, ts(ch, CHUNK)],
                                         xnb[:, h, ts(tt, P)],
                                         t_t[:, h, ts(ch, CHUNK)],
                                         start=(h == 0), stop=(h == NH - 1))
                # est = exp(st/16 + r/16 + SHIFT)  (fp8e4; max ~107 < 240)
                nc.scalar.activation(out=est[:, tt], in_=ps[:], func=AF.Exp,
                                     bias=r2[:, tt:tt + 1], scale=1.0 / 16.0)

        with _phase(nc, "colsum"):
    # ------------- softmax denominator (replicated): DoubleRow ones --------
            ps_cs = ps_big.tile([P, N], F32, tag="big")
            for tp in range(NT // 2):
                for ch in range(NCH):
                    nc.tensor.matmul(ps_cs[:, ts(ch, CHUNK)], s_ones8[:],
                                     est[:, 2 * tp:2 * tp + 2, ts(ch, CHUNK)],
                                     start=(tp == 0), stop=(tp == NT // 2 - 1),
                                     perf_mode=DR)
            # den is in [~20, 2000]: approx_fast's ~18 correct bits are far
            # beyond the fp8-softmax error floor, at ~5x less DVE time.
            recip = p_recip.tile([P, N], F32, tag="recip")
            for ch in range(NCH):
                nc.vector.reciprocal_approx_fast(
                    out=recip[:, ts(ch, CHUNK)], in_=ps_cs[:, ts(ch, CHUNK)])

        with _phase(nc, "attnv"):
    # ------- attn @ v' (DoubleRow) -> normalize -> +residual -> DMA out ----
            fin = p_fin.tile([P, NH, N], F32, tag="fin")
            for m in range(NH):
                ps = ps_big.tile([P, N], F32, tag="big")
                for tp in range(NT // 2):
                    for ch in range(NCH):
                        nc.tensor.matmul(ps[:, ts(ch, CHUNK)],
                                         vt[:, 2 * tp:2 * tp + 2, ts(m, P)],
                                         est[:, 2 * tp:2 * tp + 2, ts(ch, CHUNK)],
                                         start=(tp == 0),
                                         stop=(tp == NT // 2 - 1),
                                         perf_mode=DR)
                # chunk-granular normalize -> +residual -> DMA (short drain);
                # residual add alternates Pool/DVE so the tail drains on two
                # engines instead of serializing on Pool.
                tmp = p_tmp.tile([P, N], F32, tag="tmp")
                add_eng = (nc.vector if (m == 1 and img == IMGS - 1)
                           else nc.gpsimd)
                for ch in range(NCH):
                    nc.vector.tensor_tensor(tmp[:, ts(ch, CHUNK)],
                                            ps[:, ts(ch, CHUNK)],
                                            recip[:, ts(ch, CHUNK)], OP.mult)
                    add_eng.tensor_tensor(fin[:, m, ts(ch, CHUNK)],
                                          tmp[:, ts(ch, CHUNK)],
                                          xnfb[:, m, ts(ch, CHUNK)], OP.add)
                    nc.sync.dma_start(out_ap[img, m, :, ts(ch, CHUNK)],
                                      fin[:, m, ts(ch, CHUNK)])


def _build(reps: int = 1):
    nc = bacc.Bacc("TRN2", debug=False, num_devices=N_CORES)
    t = {}
    t["x"] = nc.dram_tensor("x", [IMGS, NH, P, N], F32, kind="ExternalInput").ap()
    t["gT"] = nc.dram_tensor("gT", [NH, P, C], F16, kind="ExternalInput").ap()
    t["huT"] = nc.dram_tensor("huT", [NH, P, C + 1], F16,
                              kind="ExternalInput").ap()
    t["gnw"] = nc.dram_tensor("gnw", [NH, P], F32, kind="ExternalInput").ap()
    t["gnbfb"] = nc.dram_tensor("gnbfb", [NH, P, 2], F32, kind="ExternalInput").ap()
    t["ind"] = nc.dram_tensor("ind", [NH, P, GROUPS], F32, kind="ExternalInput").ap()
    t["indT"] = nc.dram_tensor("indT", [GROUPS, NH, P], F32, kind="ExternalInput").ap()
    t["out"] = nc.dram_tensor("out", [IMGS, NH, P, N], F32, kind="ExternalOutput").ap()
    with tile.TileContext(nc) as tc:
        with ExitStack() as ctx:
            _emit(ctx, tc, t, reps=reps)
    nc.compile()
    return nc


def _host_inputs(x, gn_w, gn_b, qkv_w, qkv_b, out_w, out_b):
    """Build the per-core input maps (host-side weight prep)."""
    x = np.asarray(x, dtype=np.float32).reshape(B, C, N)
    gn_w = np.asarray(gn_w, dtype=np.float32)
    gn_b = np.asarray(gn_b, dtype=np.float32)
    qkv_w = np.asarray(qkv_w, dtype=np.float64)
    qkv_b = np.asarray(qkv_b, dtype=np.float64)
    out_w = np.asarray(out_w, dtype=np.float64)
    out_b = np.asarray(out_b, dtype=np.float64)

    Wq, Wk, Wv = qkv_w[:C], qkv_w[C:2 * C], qkv_w[2 * C:]
    bq, bv = qkv_b[:C], qkv_b[2 * C:]
    G = Wk.T @ Wq                       # st[m,n] = xn_m^T G xn_n
    Hu = np.concatenate([out_w @ Wv, (Wk.T @ bq)[None, :]], axis=0)  # [257,256]
    fb = (out_w @ bv + out_b).astype(np.float32)

    # lhsT/rhs layouts: [h, p, o] with o the output/free dim
    gT = np.ascontiguousarray(G.T).reshape(NH, P, C).astype(np.float16)
    huT = np.ascontiguousarray(Hu.T).reshape(NH, P, C + 1).astype(np.float16)
    gnbfb = np.stack([gn_b, gn_b + fb], axis=-1).reshape(NH, P, 2).astype(np.float32)
    gnw = gn_w.reshape(NH, P).astype(np.float32)

    # local-group indicators (4 groups per 128-channel half, identical per half)
    ind = np.zeros((NH, P, GROUPS), np.float32)
    indT = np.zeros((GROUPS, NH, P), np.float32)
    cpg = C // GROUPS  # channels per group = 32
    for h in range(NH):
        for p in range(P):
            gl = p // cpg
            ind[h, p, gl] = 1.0 / cpg
            indT[gl, h, p] = 1.0

    shared = dict(gT=gT, huT=huT, gnw=gnw, gnbfb=gnbfb, ind=ind, indT=indT)
    in_maps = []
    for core in range(N_CORES):
        xs = x[core * IMGS:(core + 1) * IMGS].reshape(IMGS, NH, P, N)
        in_maps.append(dict(shared, x=np.ascontiguousarray(xs)))
    return in_maps


_NC_CACHE = {}


def _get_nc(reps: int = 1):
    if reps not in _NC_CACHE:
        _NC_CACHE[reps] = _build(reps=reps)
    return _NC_CACHE[reps]


def kernel(x, gn_w, gn_b, qkv_w, qkv_b, out_w, out_b, _reps=1):
    nc = _get_nc(_reps)
    in_maps = _host_inputs(x, gn_w, gn_b, qkv_w, qkv_b, out_w, out_b)
    res = run_bass_kernel_spmd(nc, in_maps, core_ids=list(range(N_CORES)))
    out = np.concatenate([r["out"].reshape(IMGS, C, H, W) for r in res.results])
    kernel.last_results = res
    return out


# revision 13
# speedup vs baseline: 1.0642x; 1.0642x over previous
"""Trainium2 Bass kernel for GroupNorm -> self-attention -> proj + residual.

v2 design (per image, b=32 total, data-parallel over 8 cores):
    xn    = GroupNorm(x, 8 groups, affine)                    [c=256, n=1024]
    Host folds the projections algebraically:
      G  = Wk^T Wq          st[m,n] = xn_m^T G xn_n  (q/k matmuls fused)
      Hu = [Wo Wv; u^T]     v'[m,:] = H xn_m,  r_m = u.xn_m (u = Wk^T bq)
      fb = Wo bv + out_b    (folded into the GroupNorm residual tile)
    The per-query score-bias terms cancel in softmax; the per-key term
    r_m rides as an extra output column of the v' matmul and folds into
    the exp bias.  exp is shifted by -3 so est fits fp8e4 (max ~107<240).
    est and v' are stored fp8e4 -> attn.v and the denominator column-sum
    run as DoubleRow matmuls (2x PE rate); the output projection is gone
    (folded into v'), so attn.v produces the final pre-residual output.

Engine split: PE matmuls; ACT t-copyback + exp; DVE stats/approx-recip/
copies/psum-normalize; Pool (GPSIMD) GroupNorm applies + residual adds
(the m=1 adds ride DVE so the tail drains on two engines).  Matmul inputs
fp16 (t, xn, G, Hu) and fp8e4 (est, v'); accumulation fp32.

HW-measured notes (axon trn2, wall-clock rep-differencing):
  - baseline (q/k/v + proj matmuls, bf16, full DVE reciprocal): 172.1 us
  - this kernel: ~120-127 us per 4-image iteration; rel err 1.27e-2.
  - DVE `reciprocal` costs ~6.7 cyc/elem on HW (5x the cost model);
    `reciprocal_approx_fast` (~18 bits) measured ~5x faster end-to-end.
  - fp8 DoubleRow MM (N=512) ~234 ns vs bf16 ~252 ns per instruction
    (2.15x per unit contraction) - model undercharges DR at 107 ns.
  - For_i loop overhead (barrier + back-edge) is only ~1.7 us/iter;
    unrolling 2 bodies/iteration measured SLOWER (instruction fetch).
"""

import numpy as np
from contextlib import ExitStack

import concourse.bass as bass
import concourse.tile as tile
import concourse.mybir as mybir
from concourse import bacc
from concourse.bass import ts
from concourse.bass_utils import run_bass_kernel_spmd

P = 128
N_CORES = 8
B, C, H, W = 32, 256, 32, 32
N = H * W                      # 1024 pixels
IMGS = B // N_CORES            # 4 images per core
NH = C // P                    # 2 channel halves
NT = N // P                    # 8 pixel tiles
GROUPS = 8
EPS = 1e-5
F32 = mybir.dt.float32
F16 = mybir.dt.float16
F8 = mybir.dt.float8e4
AF = mybir.ActivationFunctionType
OP = mybir.AluOpType
DR = mybir.MatmulPerfMode.DoubleRow
CHUNK = 512                    # matmul moving free dim (one PSUM bank f32)
NCH = N // CHUNK               # 2 chunks
SHIFT = -3.0                   # exp shift: est = exp(st/16 - 3), cancels in ratio

PHASE_OF = {}


class _phase:
    """Records which instructions each phase emits (for trace attribution)."""

    def __init__(self, nc, name):
        self.nc, self.name = nc, name

    def __enter__(self):
        self.before = set(self.nc.inst_map)
        return self

    def __exit__(self, *a):
        for n in set(self.nc.inst_map) - self.before:
            PHASE_OF[n] = self.name


def _emit(ctx: ExitStack, tc: tile.TileContext, t: dict, reps: int = 1):
    nc = tc.nc

    singles = ctx.enter_context(tc.tile_pool(name="singles", bufs=1))
    p_x = ctx.enter_context(tc.tile_pool(name="p_x", bufs=3))
    p_stats = ctx.enter_context(tc.tile_pool(name="p_stats", bufs=4))
    p_xnb = ctx.enter_context(tc.tile_pool(name="p_xnb", bufs=3))
    p_xnfb = ctx.enter_context(tc.tile_pool(name="p_xnfb", bufs=3))
    p_t = ctx.enter_context(tc.tile_pool(name="p_t", bufs=3))
    p_vt = ctx.enter_context(tc.tile_pool(name="p_vt", bufs=3))
    p_r2 = ctx.enter_context(tc.tile_pool(name="p_r2", bufs=3))
    p_est = ctx.enter_context(tc.tile_pool(name="p_est", bufs=3))
    p_recip = ctx.enter_context(tc.tile_pool(name="p_recip", bufs=3))
    p_tmp = ctx.enter_context(tc.tile_pool(name="p_tmp", bufs=6))
    p_fin = ctx.enter_context(tc.tile_pool(name="p_fin", bufs=3))
    ps_big = ctx.enter_context(tc.tile_pool(name="ps_big", bufs=3, space="PSUM"))
    ps_sm = ctx.enter_context(tc.tile_pool(name="ps_sm", bufs=1, space="PSUM"))

    # ---- load constants / weights into SBUF once ----
    s_gT = singles.tile([P, NH, C], F16)
    nc.sync.dma_start(s_gT[:], t["gT"].rearrange("h p o -> p h o"))
    s_huT = singles.tile([P, NH, C + 1], F16)
    nc.sync.dma_start(s_huT[:], t["huT"].rearrange("h p o -> p h o"))
    s_gnw = singles.tile([P, NH], F32)
    nc.sync.dma_start(s_gnw[:], t["gnw"].rearrange("h p -> p h"))
    s_gnbfb = singles.tile([P, NH, 2], F32)  # col0 = gn_b, col1 = gn_b + fb
    nc.sync.dma_start(s_gnbfb[:], t["gnbfb"].rearrange("h p k -> p h k"))
    s_ind = singles.tile([P, NH, GROUPS], F32)
    nc.sync.dma_start(s_ind[:], t["ind"].rearrange("h p g -> p h g"))
    s_indT = singles.tile([GROUPS, NH, P], F32)
    nc.sync.dma_start(s_indT[:], t["indT"])
    s_ones8 = singles.tile([P, 2, P], F8)
    nc.vector.memset(s_ones8[:], 1.0)
    s_ones16 = singles.tile([P, P], F16)
    nc.vector.memset(s_ones16[:], 1.0)

    # PE warmup: dense dummy matmuls during the GroupNorm head so the HAM
    # clock-gate reaches 8/8 before the real matmuls start (HW-only effect).
    ps_w = ps_sm.tile([P, C + 1], F32, tag="vt")
    for _ in range(10):
        nc.tensor.matmul(ps_w[:, 0:C], s_ones16[:], s_gT[:, 0, :],
                         start=True, stop=True)
    w_sink = p_stats.tile([1, 1], F32, tag="wsink")
    nc.vector.tensor_copy(w_sink[:], ps_w[0:1, 0:1])

    x_ap = t["x"]       # [IMGS, NH, P, N]
    out_ap = t["out"]   # [IMGS, NH, P, N]

    def body():
        _body(nc, tc, t, pools)

    pools = dict(p_x=p_x, p_stats=p_stats, p_xnb=p_xnb, p_xnfb=p_xnfb,
                 p_t=p_t, p_vt=p_vt, p_r2=p_r2, p_est=p_est,
                 p_recip=p_recip, p_tmp=p_tmp, p_fin=p_fin, ps_big=ps_big,
                 ps_sm=ps_sm, s_gT=s_gT, s_huT=s_huT, s_gnw=s_gnw,
                 s_gnbfb=s_gnbfb, s_ind=s_ind, s_indT=s_indT,
                 s_ones8=s_ones8, x_ap=x_ap, out_ap=out_ap)

    # One body per hardware-loop iteration. (Unrolling 2 bodies/iteration was
    # tried and measured SLOWER on HW — the doubled instruction stream costs
    # more in sequencer fetch than the amortized barrier saves.)
    if reps > 1:
        with tc.For_i(0, reps, 1, hint_engines=(mybir.EngineType.PE,)):
            body()
    else:
        body()


def _body(nc, tc, t, pl):
    p_x, p_stats, p_xnb = pl["p_x"], pl["p_stats"], pl["p_xnb"]
    p_xnfb, p_t, p_vt = pl["p_xnfb"], pl["p_t"], pl["p_vt"]
    p_r2, p_est, p_recip = pl["p_r2"], pl["p_est"], pl["p_recip"]
    p_tmp, p_fin, ps_big, ps_sm = pl["p_tmp"], pl["p_fin"], pl["ps_big"], pl["ps_sm"]
    s_gT, s_huT, s_gnw = pl["s_gT"], pl["s_huT"], pl["s_gnw"]
    s_gnbfb, s_ind, s_indT = pl["s_gnbfb"], pl["s_ind"], pl["s_indT"]
    s_ones8, x_ap, out_ap = pl["s_ones8"], pl["x_ap"], pl["out_ap"]

    for img in range(IMGS):
        with _phase(nc, "gn"), tc.high_priority(offset=120):
    # ---------------- GroupNorm (fully per-half: groups never span halves) ---
    # high_priority: img i+1's GN (DMA/stats/applies) must not queue behind
    # img i's attention epilogue on Pool/DVE, or the next image's matmuls
    # stall waiting for xnb.
            x_t = p_x.tile([P, NH, N], F32, tag="x")
            xnb = p_xnb.tile([P, NH, N], F16, tag="xnb")
            xnfb = p_xnfb.tile([P, NH, N], F32, tag="xnfb")
            for h in range(NH):
                for s in range(2):
                    nc.sync.dma_start(x_t[:, h, ts(s, CHUNK)],
                                      x_ap[img, h, :, ts(s, CHUNK)])

                # per-channel mean / E[x^2] via bn_stats (free dim cap 512)
                st6 = p_stats.tile([P, 2, 6], F32, tag="st6")
                xv = x_t[:, h].rearrange("p (s f) -> p s f", f=512)
                for s in range(2):
                    nc.vector.bn_stats(out=st6[:, s, :], in_=xv[:, s, :])
                mv = p_stats.tile([P, 2], F32, tag="mv")
                nc.vector.bn_aggr(out=mv[:], in_=st6[:])
                mm = p_stats.tile([P, 2], F32, tag="mm")  # (mean, E[x^2])
                nc.vector.tensor_copy(mm[:, 0:1], mv[:, 0:1])
                nc.vector.tensor_tensor(mm[:, 1:2], mv[:, 0:1], mv[:, 0:1], OP.mult)
                nc.vector.tensor_tensor(mm[:, 1:2], mm[:, 1:2], mv[:, 1:2], OP.add)

                # this half's 4 group stats: [4, 2] = ind_h.T @ mm
                psg = ps_sm.tile([4, 2], F32, tag="gn")
                nc.tensor.matmul(psg[:], s_ind[:, h, :4], mm[:],
                                 start=True, stop=True)
                grp = p_stats.tile([4, 2], F32, tag="grp")  # (mu, rstd)
                nc.vector.tensor_copy(grp[:, 0:1], psg[:, 0:1])
                nc.vector.tensor_copy(grp[:, 1:2], psg[:, 1:2])
                v = p_stats.tile([4, 3], F32, tag="musq")  # var+eps, s, t
                nc.vector.tensor_tensor(v[:, 1:2], grp[:, 0:1], grp[:, 0:1], OP.mult)
                nc.vector.tensor_tensor(v[:, 0:1], grp[:, 1:2], v[:, 1:2], OP.subtract)
                nc.vector.tensor_scalar(out=v[:, 0:1], in0=v[:, 0:1], scalar1=EPS,
                                        scalar2=None, op0=OP.add)
                # rstd = 1/sqrt(v) by Newton on sqrt from s0=1 (group var ~ 1),
                # all on DVE — keeps ACT's table set pinned to exp.
                # s <- 0.5*(s + v/s), twice; then rstd = 1/s.
                nc.vector.tensor_scalar(out=v[:, 1:2], in0=v[:, 0:1], scalar1=1.0,
                                        scalar2=0.5, op0=OP.add, op1=OP.mult)
                for _ in range(2):
                    nc.vector.reciprocal(v[:, 2:3], v[:, 1:2])
                    nc.vector.tensor_tensor(v[:, 2:3], v[:, 0:1], v[:, 2:3], OP.mult)
                    nc.vector.tensor_tensor(v[:, 1:2], v[:, 1:2], v[:, 2:3], OP.add)
                    nc.vector.tensor_scalar(out=v[:, 1:2], in0=v[:, 1:2],
                                            scalar1=0.5, scalar2=None, op0=OP.mult)
                nc.vector.reciprocal(grp[:, 1:2], v[:, 1:2])

                # broadcast 4 group (mu, rstd) to this half's 128 channels
                psb = ps_sm.tile([P, 2], F32, tag="gn")
                nc.tensor.matmul(psb[:], s_indT[:4, h, :], grp[:],
                                 start=True, stop=True)
                ab = p_stats.tile([P, 3], F32, tag="ab")  # a, b, b+fb
                a = ab[:, 0:1]
                nc.vector.tensor_tensor(a, psb[:, 1:2], s_gnw[:, h:h + 1], OP.mult)
                mua = ab[:, 1:2]
                nc.vector.tensor_tensor(mua, psb[:, 0:1], a, OP.mult)
                # b = gn_b - mu*a ; b_fb = (gn_b + fb) - mu*a
                nc.vector.tensor_tensor(ab[:, 2:3], s_gnbfb[:, h, 1:2], mua, OP.subtract)
                nc.vector.tensor_tensor(mua, s_gnbfb[:, h, 0:1], mua, OP.subtract)

                # apply on GPSIMD: xnb = fp16(x*a+b); xnfb = f32(x*a+(b+fb))
                # (per-chunk so downstream matmuls can start on chunk 0)
                for s in range(2):
                    nc.gpsimd.tensor_scalar(out=xnb[:, h, ts(s, CHUNK)],
                                            in0=x_t[:, h, ts(s, CHUNK)],
                                            scalar1=ab[:, 0:1], scalar2=ab[:, 1:2],
                                            op0=OP.mult, op1=OP.add)
                for s in range(2):
                    nc.gpsimd.tensor_scalar(out=xnfb[:, h, ts(s, CHUNK)],
                                            in0=x_t[:, h, ts(s, CHUNK)],
                                            scalar1=ab[:, 0:1], scalar2=ab[:, 2:3],
                                            op0=OP.mult, op1=OP.add)

        with _phase(nc, "t"):
    # ---------------- t = G @ xn  (q/k fused; [c, n] layout) ----------------
            t_t = p_t.tile([P, NH, N], F16, tag="t")
            for j in range(NH):
                ps = ps_big.tile([P, N], F32, tag="big")
                for h in range(NH):
                    for ch in range(NCH):
                        nc.tensor.matmul(ps[:, ts(ch, CHUNK)],
                                         s_gT[:, h, ts(j, P)],
                                         xnb[:, h, ts(ch, CHUNK)],
                                         start=(h == 0), stop=(h == NH - 1))
                nc.scalar.activation(out=t_t[:, j], in_=ps[:], func=AF.Copy)

        with _phase(nc, "vt"):
    # v' in [n, c] layout via lhsT=xnb; extra col 256 = r_m = u.xn_m
            vt = p_vt.tile([P, NT, C], F8, tag="vt")
            r2 = p_r2.tile([P, NT], F32, tag="r2")   # exp bias: r/16 + SHIFT
            for tt in range(NT):
                ps2 = ps_sm.tile([P, C + 1], F32, tag="vt")
                for h in range(NH):
                    nc.tensor.matmul(ps2[:], xnb[:, h, ts(tt, P)],
                                     s_huT[:, h, :],
                                     start=(h == 0), stop=(h == NH - 1))
                nc.vector.tensor_copy(out=vt[:, tt], in_=ps2[:, 0:C])
                nc.vector.tensor_scalar(out=r2[:, tt:tt + 1],
                                        in0=ps2[:, C:C + 1],
                                        scalar1=1.0 / 16.0, scalar2=SHIFT,
                                        op0=OP.mult, op1=OP.add)

        with _phase(nc, "scores"):
    # ---------------- scores^T (st[m,n] = xn_m.G.xn_n) + exp ----------------
            est = p_est.tile([P, NT, N], F8, tag="est")
            for tt in range(NT):
                ps = ps_big.tile([P, N], F32, tag="big")
                for h in range(NH):
                    for ch in range(NCH):
                        nc.tensor.matmul(ps[:, ts(ch, CHUNK)],
                                         xnb[:, h, ts(tt, P)],
                                         t_t[:, h, ts(ch, CHUNK)],
                                         start=(h == 0), stop=(h == NH - 1))
                # est = exp(st/16 + r/16 + SHIFT)  (fp8e4; max ~107 < 240)
                nc.scalar.activation(out=est[:, tt], in_=ps[:], func=AF.Exp,
                                     bias=r2[:, tt:tt + 1], scale=1.0 / 16.0)

        with _phase(nc, "colsum"):
    # ------------- softmax denominator (replicated): DoubleRow ones --------
            ps_cs = ps_big.tile([P, N], F32, tag="big")
            for tp in range(NT // 2):
                for ch in range(NCH):
                    nc.tensor.matmul(ps_cs[:, ts(ch, CHUNK)], s_ones8[:],
                                     est[:, 2 * tp:2 * tp + 2, ts(ch, CHUNK)],
                                     start=(tp == 0), stop=(tp == NT // 2 - 1),
                                     perf_mode=DR)
            # den is in [~20, 2000]: approx_fast's ~18 correct bits are far
            # beyond the fp8-softmax error floor, at ~5x less DVE time.
            recip = p_recip.tile([P, N], F32, tag="recip")
            for ch in range(NCH):
                nc.vector.reciprocal_approx_fast(
                    out=recip[:, ts(ch, CHUNK)], in_=ps_cs[:, ts(ch, CHUNK)])

        with _phase(nc, "attnv"):
    # ------- attn @ v' (DoubleRow) -> normalize -> +residual -> DMA out ----
            fin = p_fin.tile([P, NH, N], F32, tag="fin")
            for m in range(NH):
                ps = ps_big.tile([P, N], F32, tag="big")
                for tp in range(NT // 2):
                    for ch in range(NCH):
                        nc.tensor.matmul(ps[:, ts(ch, CHUNK)],
                                         vt[:, 2 * tp:2 * tp + 2, ts(m, P)],
                                         est[:, 2 * tp:2 * tp + 2, ts(ch, CHUNK)],
                                         start=(tp == 0),
                                         stop=(tp == NT // 2 - 1),
                                         perf_mode=DR)
                # chunk-granular normalize -> +residual -> DMA (short drain);
                # residual add alternates Pool/DVE so the tail drains on two
                # engines instead of serializing on Pool.
                tmp = p_tmp.tile([P, N], F32, tag="tmp")
                add_eng = nc.gpsimd if m == 0 else nc.vector
                for ch in range(NCH):
                    nc.vector.tensor_tensor(tmp[:, ts(ch, CHUNK)],
                                            ps[:, ts(ch, CHUNK)],
                                            recip[:, ts(ch, CHUNK)], OP.mult)
                    add_eng.tensor_tensor(fin[:, m, ts(ch, CHUNK)],
                                          tmp[:, ts(ch, CHUNK)],
                                          xnfb[:, m, ts(ch, CHUNK)], OP.add)
                    nc.sync.dma_start(out_ap[img, m, :, ts(ch, CHUNK)],
                                      fin[:, m, ts(ch, CHUNK)])


def _build(reps: int = 1):
    nc = bacc.Bacc("TRN2", debug=False, num_devices=N_CORES)
    t = {}
    t["x"] = nc.dram_tensor("x", [IMGS, NH, P, N], F32, kind="ExternalInput").ap()
    t["gT"] = nc.dram_tensor("gT", [NH, P, C], F16, kind="ExternalInput").ap()
    t["huT"] = nc.dram_tensor("huT", [NH, P, C + 1], F16,
                              kind="ExternalInput").ap()
    t["gnw"] = nc.dram_tensor("gnw", [NH, P], F32, kind="ExternalInput").ap()
    t["gnbfb"] = nc.dram_tensor("gnbfb", [NH, P, 2], F32, kind="ExternalInput").ap()
    t["ind"] = nc.dram_tensor("ind", [NH, P, GROUPS], F32, kind="ExternalInput").ap()
    t["indT"] = nc.dram_tensor("indT", [GROUPS, NH, P], F32, kind="ExternalInput").ap()
    t["out"] = nc.dram_tensor("out", [IMGS, NH, P, N], F32, kind="ExternalOutput").ap()
    with tile.TileContext(nc) as tc:
        with ExitStack() as ctx:
            _emit(ctx, tc, t, reps=reps)
    nc.compile()
    return nc


def _host_inputs(x, gn_w, gn_b, qkv_w, qkv_b, out_w, out_b):
    """Build the per-core input maps (host-side weight prep)."""
    x = np.asarray(x, dtype=np.float32).reshape(B, C, N)
    gn_w = np.asarray(gn_w, dtype=np.float32)
    gn_b = np.asarray(gn_b, dtype=np.float32)
    qkv_w = np.asarray(qkv_w, dtype=np.float64)
    qkv_b = np.asarray(qkv_b, dtype=np.float64)
    out_w = np.asarray(out_w, dtype=np.float64)
    out_b = np.asarray(out_b, dtype=np.float64)

    Wq, Wk, Wv = qkv_w[:C], qkv_w[C:2 * C], qkv_w[2 * C:]
    bq, bv = qkv_b[:C], qkv_b[2 * C:]
    G = Wk.T @ Wq                       # st[m,n] = xn_m^T G xn_n
    Hu = np.concatenate([out_w @ Wv, (Wk.T @ bq)[None, :]], axis=0)  # [257,256]
    fb = (out_w @ bv + out_b).astype(np.float32)

    # lhsT/rhs layouts: [h, p, o] with o the output/free dim
    gT = np.ascontiguousarray(G.T).reshape(NH, P, C).astype(np.float16)
    huT = np.ascontiguousarray(Hu.T).reshape(NH, P, C + 1).astype(np.float16)
    gnbfb = np.stack([gn_b, gn_b + fb], axis=-1).reshape(NH, P, 2).astype(np.float32)
    gnw = gn_w.reshape(NH, P).astype(np.float32)

    # local-group indicators (4 groups per 128-channel half, identical per half)
    ind = np.zeros((NH, P, GROUPS), np.float32)
    indT = np.zeros((GROUPS, NH, P), np.float32)
    cpg = C // GROUPS  # channels per group = 32
    for h in range(NH):
        for p in range(P):
            gl = p // cpg
            ind[h, p, gl] = 1.0 / cpg
            indT[gl, h, p] = 1.0

    shared = dict(gT=gT, huT=huT, gnw=gnw, gnbfb=gnbfb, ind=ind, indT=indT)
    in_maps = []
    for core in range(N_CORES):
        xs = x[core * IMGS:(core + 1) * IMGS].reshape(IMGS, NH, P, N)
        in_maps.append(dict(shared, x=np.ascontiguousarray(xs)))
    return in_maps


_NC_CACHE = {}


def _get_nc(reps: int = 1):
    if reps not in _NC_CACHE:
        _NC_CACHE[reps] = _build(reps=reps)
    return _NC_CACHE[reps]


def kernel(x, gn_w, gn_b, qkv_w, qkv_b, out_w, out_b, _reps=1):
    nc = _get_nc(_reps)
    in_maps = _host_inputs(x, gn_w, gn_b, qkv_w, qkv_b, out_w, out_b)
    res = run_bass_kernel_spmd(nc, in_maps, core_ids=list(range(N_CORES)))
    out = np.concatenate([r["out"].reshape(IMGS, C, H, W) for r in res.results])
    kernel.last_results = res
    return out
